# revision 6
# baseline (speedup 1.0000x reference)
"""Trainium2 Bass kernel for a 3-layer bidirectional GRU + dense sigmoid head.

Problem: B=256, T=512, D=256, H=128 (Keras reset_after=True, gate order z,r,h).
Sharding: data-parallel over batch, 32 examples per core on 8 NeuronCores.

Per-core design (gate-partition layout, everything [128(h-dim), cols]):
- Input projections (xp = x @ W + b) are computed as chunked GEMMs whose
  outputs land directly in PSUM banks; the sequential scan's recurrence
  matmuls then accumulate on top of the same PSUM columns.
- Critical-path trick: the new state h_t = z*h + (1-z)*hh is kept as the
  SUM of two pieces q_t = (1-z)*hh (tanh path, critical) and p_t = z*h
  (shadow path). The next step's recurrence matmul U @ h_t is computed as
  U @ q_t + U @ p_t accumulated in PSUM, so h_t never needs materializing
  on the critical path; the materialized h (for the next layer / final
  head) is produced by the Pool engine in the shadow of the tanh path.
- Shadow elementwise ops (w = 1-z, p = z*h, h = q+p) run on the Pool
  engine; the critical chain PE->ACT(sig)->DVE(r*rh)->DVE(+xh)->ACT(tanh)
  ->DVE(q) stays on the fastest engines.
- Forward and backward chains are interleaved (independent recurrences).
- matmuls run in float32r (relaxed fp32), accumulation in fp32 PSUM.
"""

from contextlib import ExitStack

import numpy as np

import concourse.bass as bass
from concourse import bacc
import concourse.mybir as mybir
import concourse.tile as tile
from concourse.bass_utils import run_bass_kernel_spmd

H = 128
D_IN = 256
N_CORES = 8
F32 = mybir.dt.float32
F32R = mybir.dt.float32r
AF = mybir.ActivationFunctionType
ALU = mybir.AluOpType


def _r(ap):
    return ap.bitcast(F32R)


def build_gru(nc, B, T, L, GRP, has_bias, has_bhh):
    """Emit the full GRU program into `nc`."""
    NG = T // GRP
    assert T % GRP == 0

    # packed weights: cols [w | u | wd | bias(row0) | bhh(rows0-1)]
    CW = L * 2 * 2 * 3 * H          # 4608
    CU = L * 2 * 3 * H              # 2304
    c_u = CW
    c_wd = CW + CU
    c_bias = c_wd + 2
    c_bhh = c_bias + CU
    c_ones = c_bhh + L * H
    c_ind2 = c_ones + GRP * B
    c_h0 = c_ind2 + 2 * B
    C = c_h0 + 2 * B
    x = nc.dram_tensor("x", [D_IN, T * B], F32R, kind="ExternalInput")
    wpack = nc.dram_tensor("wpack", [H, C], F32R, kind="ExternalInput")
    y = nc.dram_tensor("y", [1, B], F32, kind="ExternalOutput")

    with tile.TileContext(nc) as tc, ExitStack() as ctx:
        const = ctx.enter_context(tc.tile_pool(name="const", bufs=1))
        rhsp = ctx.enter_context(tc.tile_pool(name="rhsp", bufs=3))
        outp = ctx.enter_context(tc.tile_pool(name="outp", bufs=3))
        stepp = ctx.enter_context(tc.tile_pool(name="stepp", bufs=4))
        psum = ctx.enter_context(tc.tile_pool(name="psum", bufs=1,
                                              space="PSUM"))
        pscr = ctx.enter_context(tc.tile_pool(name="pscr", bufs=2,
                                              space="PSUM"))
        dramp = ctx.enter_context(tc.tile_pool(name="dramp", bufs=1,
                                               space="DRAM"))

        # inter-layer hidden-sequence buffers (Tile-tracked DRAM)
        seqs = []
        for p in "AB":
            sf = dramp.tile([H, T * B], F32R, name=f"seq{p}f", tag=f"seq{p}f")
            sb = dramp.tile([H, T * B], F32R, name=f"seq{p}b", tag=f"seq{p}b")
            seqs.append((sf, sb))

        # ---- preload all weights with a single contiguous DMA ----
        pk = const.tile([H, C], F32R)
        nc.sync.dma_start(out=pk, in_=wpack[:])

        def w_ap(l, d, k, gi):
            c = ((l * 2 + d) * 2 + k) * 3 * H + gi * H
            return pk[:, c:c + H]

        def u_ap(l, d, gi):
            c = c_u + (l * 2 + d) * 3 * H + gi * H
            return pk[:, c:c + H]

        def wd_ap(d):
            return pk[:, c_wd + d:c_wd + d + 1]

        def bias_ap(l, d, gi):
            c = c_bias + (l * 2 + d) * 3 * H + gi * H
            return pk[0:1, c:c + H]

        def bhh_ap(l):
            return pk[0:2, c_bhh + l * H:c_bhh + (l + 1) * H]

        h0_sb = pk[:, c_h0:c_h0 + 2 * B].rearrange("p (d b) -> p d b", d=2)
        ones_sb = pk[0:1, c_ones:c_ones + GRP * B]
        ind2_sb = pk[0:2, c_ind2:c_ind2 + 2 * B]

        # recurrence carry between steps/groups: (q, p, hmat_ap) or None at
        # a layer start (h0 = zeros).
        carry = None
        outbuf = None

        def pair2(tile4, cf, cb):
            """[H, 2, B] view of a [H, 2, GRP, B] tile: fwd half at column
            cf, bwd half at column cb (asymmetric two-range AP)."""
            ps = tile4.ap[0][0]
            return bass.AP(tensor=tile4.tensor,
                           offset=tile4.offset + cf * B,
                           ap=[[ps, H], [(GRP + cb - cf) * B, 2], [1, B]])

        def zr_pair(zrb_t, cf, cb):
            """[H, 2(dir), 2(gate), B] view of zrb [H, 4, GRP, B]: fwd
            gates at column cf, bwd gates at column cb."""
            ps = zrb_t.ap[0][0]
            return bass.AP(tensor=zrb_t.tensor,
                           offset=zrb_t.offset + cf * B,
                           ap=[[ps, H], [(2 * GRP + cb - cf) * B, 2],
                               [GRP * B, 2], [1, B]])

        for l in range(L):
            for g in range(NG):
                # All DRAM traffic is t-ascending (contiguous DMA): the bwd
                # group g covers t in [T-GRP*(g+1), T-GRP*g) and the bwd
                # scan simply indexes its PSUM/SBUF columns in reverse.
                # ---- rhs tiles (moving operand of the xp GEMM) ----
                rhs = {}
                for d, dn in ((0, "f"), (1, "b")):
                    t_lo = GRP * g if d == 0 else T - GRP * (g + 1)
                    for k in range(2):
                        rt = rhsp.tile([H, GRP, B], F32R, tag=f"rhs{dn}{k}",
                                       name=f"rhs_{dn}{k}_{l}_{g}")
                        if l == 0:
                            s_fb = x[:][k * H:(k + 1) * H, :]
                        else:
                            s_fb = seqs[(l - 1) % 2][k]  # k0=fwd, k1=bwd half
                        src = s_fb.rearrange("p (t b) -> p t b", b=B)[
                            :, t_lo:t_lo + GRP, :]
                        nc.sync.dma_start(out=rt, in_=src)
                        rhs[(d, k)] = rt

                # ---- PSUM banks ----
                zrb = psum.tile([H, 4, GRP, B], F32, tag="zrb",
                                name=f"zrb_{l}_{g}")
                xph = psum.tile([H, 2, GRP, B], F32, tag="xph",
                                name=f"xph_{l}_{g}")

                # ---- xp GEMM: accumulate x @ W (+ b) into the banks ----
                for d in (0, 1):
                    for gi in range(3):
                        out_ap = (zrb[:, 2 * d + gi, :, :] if gi < 2
                                  else xph[:, d, :, :])
                        for k in range(2):
                            nc.tensor.matmul(
                                out_ap,
                                _r(w_ap(l, d, k, gi)),
                                _r(rhs[(d, k)]),
                                start=(k == 0), stop=False,
                                skip_group_check=True)
                        if has_bias:
                            nc.tensor.matmul(
                                out_ap,
                                _r(bias_ap(l, d, gi)),
                                _r(ones_sb),
                                start=False, stop=False,
                                skip_group_check=True)

                outbuf = outp.tile([H, 2, GRP, B], F32R, tag="outbuf",
                                   name=f"outbuf_{l}_{g}")

                # ---- the sequential scan: GRP fwd+bwd step-pairs ----
                # fwd step tl uses column tl; bwd step tl uses GRP-1-tl.
                for tl in range(GRP):
                    cb = GRP - 1 - tl
                    cols = (tl, cb)
                    if carry is None:
                        q_prev = p_prev = h0_sb[:, :, :]     # [H, 2, B] zeros
                        hmat = h0_sb[:, :, :]
                    else:
                        q_prev, p_prev, hmat = carry

                    scratch = pscr.tile([H, 2, B], F32, tag="scratch",
                                        name=f"scr_{l}_{g}_{tl}")

                    # ---- PE: U @ h_prev = U @ p_prev + U @ q_prev ----
                    # p is ready early (shadow of prev step); q is the late
                    # critical piece. z,r gates first so sigma starts ASAP;
                    # h-gate matmuls (only needed by the tanh path) last.
                    for rhs_prev, is_q in ((p_prev, False), (q_prev, True)):
                        for gi in (0, 1):
                            for d in (0, 1):
                                nc.tensor.matmul(
                                    zrb[:, 2 * d + gi, cols[d], :],
                                    _r(u_ap(l, d, gi)),
                                    _r(rhs_prev[:, d, :]),
                                    start=False, stop=is_q,
                                    skip_group_check=True)
                        for d in (0, 1):
                            nc.tensor.matmul(
                                scratch[:, d, :],
                                _r(u_ap(l, d, 2)),
                                _r(rhs_prev[:, d, :]),
                                start=(not is_q and d == 0), stop=is_q,
                                skip_group_check=True)
                    if has_bhh:
                        nc.tensor.matmul(
                            scratch[:, :, :], _r(bhh_ap(l)),
                            _r(ind2_sb), start=False, stop=True,
                            skip_group_check=True)

                    zrout = stepp.tile([H, 2, 2, B], F32, tag="zrout",
                                       name=f"zrout_{l}_{g}_{tl}")
                    w = stepp.tile([H, 2, B], F32, tag="w",
                                   name=f"w_{l}_{g}_{tl}")
                    pnew = stepp.tile([H, 2, B], F32R, tag="p",
                                      name=f"p_{l}_{g}_{tl}")
                    tt = stepp.tile([H, 2, B], F32, tag="tt",
                                    name=f"tt_{l}_{g}_{tl}")
                    arg = stepp.tile([H, 2, B], F32, tag="arg",
                                     name=f"arg_{l}_{g}_{tl}")
                    hh = stepp.tile([H, 2, B], F32, tag="hh",
                                    name=f"hh_{l}_{g}_{tl}")
                    qnew = stepp.tile([H, 2, B], F32R, tag="q",
                                      name=f"q_{l}_{g}_{tl}")

                    # ACT: one sigma over both dirs x both gates (PSUM read)
                    nc.scalar.activation(
                        zrout[:, :, :, :], zr_pair(zrb, tl, cb), AF.Sigmoid)
                    # Pool shadows: w = 1-z ; p = z * h_prev
                    nc.gpsimd.tensor_scalar(
                        w[:, :, :], zrout[:, :, 0, :], 1.0, -1.0,
                        ALU.subtract, ALU.mult)
                    nc.gpsimd.tensor_mul(
                        pnew[:, :, :], zrout[:, :, 0, :], hmat)
                    # DVE critical path: tt = rec_h * r ; arg = tt + xp_h
                    nc.vector.tensor_mul(
                        tt[:, :, :], scratch[:, :, :], zrout[:, :, 1, :])
                    nc.vector.tensor_add(
                        arg[:, :, :], tt[:, :, :], pair2(xph, tl, cb))
                    nc.scalar.activation(hh[:, :, :], arg[:, :, :], AF.Tanh)
                    nc.vector.tensor_mul(
                        qnew[:, :, :], w[:, :, :], hh[:, :, :])
                    # Pool shadow: materialize h_t = q + p for the next
                    # layer's sequence (and next step's p computation).
                    hm = pair2(outbuf, tl, cb)
                    nc.gpsimd.tensor_add(hm, qnew[:, :, :], pnew[:, :, :])
                    carry = (qnew, pnew, hm)

                # ---- store the group's hidden states (layers 0..L-2) ----
                if l < L - 1:
                    sf, sb = seqs[l % 2]
                    nc.sync.dma_start(
                        out=sf.rearrange("p (t b) -> p t b", b=B)[
                            :, GRP * g:GRP * (g + 1), :],
                        in_=outbuf[:, 0, :, :])
                    t_lo_b = T - GRP * (g + 1)
                    nc.sync.dma_start(
                        out=sb.rearrange("p (t b) -> p t b", b=B)[
                            :, t_lo_b:t_lo_b + GRP, :],
                        in_=outbuf[:, 1, :, :])
            carry = None  # h resets between layers

        # ---- dense head on the final states of the last group ----
        py = pscr.tile([1, B], F32, tag="scratch", name="py")
        nc.tensor.matmul(py, _r(wd_ap(0)),
                         _r(outbuf[:, 0, GRP - 1, :]),
                         start=True, stop=False, skip_group_check=True)
        nc.tensor.matmul(py, _r(wd_ap(1)),
                         _r(outbuf[:, 1, 0, :]),
                         start=False, stop=True, skip_group_check=True)
        y_sb = const.tile([1, B], F32)
        nc.scalar.activation(y_sb, py, AF.Sigmoid)
        nc.sync.dma_start(out=y[:], in_=y_sb)


def _prep_host(Ws, Us, bs, Wd, L, GRP, B_loc):
    """Pack all replicated weights into one [128, C] array (single DMA)."""
    Ws = np.asarray(Ws, np.float32)
    Us = np.asarray(Us, np.float32)
    bs = np.asarray(bs, np.float32)
    Wd = np.asarray(Wd, np.float32)
    has_bias = bool(np.any(bs != 0))
    has_bhh = bool(np.any(bs[:, :, 1, 2 * H:] != 0))
    CW = L * 2 * 2 * 3 * H
    CU = L * 2 * 3 * H
    GRPB = GRP * B_loc
    C = CW + CU + 2 + CU + L * H + GRPB + 4 * B_loc
    pack = np.zeros((H, C), np.float32)
    c_ones = CW + 2 * CU + 2 + L * H
    pack[0, c_ones:c_ones + GRPB] = 1.0           # ones row for bias MMs
    pack[0, c_ones + GRPB:c_ones + GRPB + B_loc] = 1.0        # ind2 row 0
    pack[1, c_ones + GRPB + B_loc:c_ones + GRPB + 2 * B_loc] = 1.0
    # w: [l, d, k(row-chunk), p(row within chunk), h] -> [p, (l d k h)]
    pack[:, :CW] = (Ws.reshape(L, 2, 2, H, 3 * H)
                    .transpose(3, 0, 1, 2, 4).reshape(H, CW))
    pack[:, CW:CW + CU] = (Us.transpose(2, 0, 1, 3).reshape(H, CU))
    pack[:, CW + CU] = Wd[0:H, 0]
    pack[:, CW + CU + 1] = Wd[H:2 * H, 0]
    if has_bias:
        bsum = bs[:, :, 0, :].copy()               # b_i everywhere
        bsum[:, :, :2 * H] += bs[:, :, 1, :2 * H]  # + b_h on z,r
        pack[0, CW + CU + 2:CW + 2 * CU + 2] = bsum.reshape(-1)
    if has_bhh:
        cb = CW + 2 * CU + 2
        pack[0:2, cb:cb + L * H] = np.transpose(
            bs[:, :, 1, 2 * H:], (1, 0, 2)).reshape(2, L * H)
    return {"wpack": pack}, has_bias, has_bhh


def run_gru(x, Ws, Us, bs, Wd, bd, n_cores=N_CORES, L=3, GRP=16, trace=False):
    x = np.ascontiguousarray(np.asarray(x, np.float32))
    B_full, T, _ = x.shape
    B_loc = B_full // n_cores
    common, has_bias, has_bhh = _prep_host(Ws, Us, bs, Wd, L, GRP, B_loc)

    nc = bacc.Bacc()
    build_gru(nc, B_loc, T, L, GRP, has_bias, has_bhh)
    nc.compile()

    in_maps = []
    for c in range(n_cores):
        m = dict(common)
        xs = x[c * B_loc:(c + 1) * B_loc]          # [B_loc, T, D]
        m["x"] = np.ascontiguousarray(
            xs.transpose(2, 1, 0).reshape(D_IN, T * B_loc))
        in_maps.append(m)

    res = run_bass_kernel_spmd(nc, in_maps, core_ids=list(range(n_cores)),
                               trace=trace)
    parts = [res.results[c]["y"][0] for c in range(n_cores)]
    out = np.concatenate(parts).reshape(B_full, 1).astype(np.float32)
    return out, res


def kernel(x, Ws, Us, bs, Wd, bd):
    bd = np.asarray(bd, np.float32).reshape(-1)
    out, _ = run_gru(x, Ws, Us, bs, Wd, bd)
    if np.any(bd != 0):
        # bd is zero in the spec; if not, fold it in via logit shift
        p = np.clip(np.float64(out), 1e-12, 1 - 1e-12)
        out = (1.0 / (1.0 + np.exp(-(np.log(p / (1 - p)) + bd[0]))))
    return np.asarray(out, np.float32)
